# revision 82
# baseline (speedup 1.0000x reference)
"""Pointer-generator attention kernel for 8 TRN2 NeuronCores.

Computation (per batch b):
    enc_feat = h[b] @ W_h.T                       # [T, N]
    att      = enc_feat + dec_fea[b] + cov[b,:,None] * W_c
    scores   = tanh(att) @ v                      # [T]
    attn     = exp(scores) * mask / sum(...)      # [T]
    c_t      = attn @ h[b]                        # [N]
    cov_new  = cov + attn

Sharding: data-parallel over batch, 8 batches per core, no collectives.

Device-side layout (per core):
    hT [8, N, T] -- h transposed per batch, so the contraction dim n sits
    on SBUF partitions for the main matmul AND the t axis is the free dim
    for the pass-B reduce.
    Main matmul precision is a hybrid K-split: contraction chunks 0..3 run
    as fp8e4 DoubleRow matmuls (both operands fp8, K=256 per instruction,
    2x PE throughput), chunks 4..7 in bf16; fp32 PSUM accumulation.  The
    split is accuracy-bound: measured max rel err 1.87e-2 of the 2e-2
    budget, incl. the scaled-fp8 dec_W (pure fp8 would be >2.2e-2).  h
    ships twice: fp8 chunks 0..3
    (matmul) + full bf16 (bf16 matmul chunks + the pass-B reduce), with
    the pass-B-only chunks at lowest DMA priority and h prefetched two
    batches deep.
    att tiles [m=128, t=1024]: lhsT = W_hT chunk (stationary), rhs = hT.
    cov term via DVE STT in place on PSUM (cov rows pre-broadcast across
    partitions on the host), dec_fea folded into the tanh bias on ScalarE,
    v-dot as M=1 matmuls on PE, softmax on single-partition rows (exp has
    no overflow risk: |score| <= ||v||_1 ~ 26), pass B as fused
    multiply+reduce on VectorE over the resident hT tiles.
    Last batch's pass B runs on the PE instead (DVE would backlog the
    final softmax): the unnormalized exp row is transposed into [128, 8]
    columns by PE transpose matmuls (mask folded in the columnar copy,
    1/sum folded into the psum eviction) and contracted against natural-
    layout h tiles; warm-up matmuls pinned to the last att tiles keep the
    PE clock hot through that window.
"""

import os
import sys

import numpy as np

sys.path.insert(0, "/opt/trn_rl_repo")

import concourse.bass as bass  # noqa: E402
import concourse.tile as tile  # noqa: E402
from concourse import mybir  # noqa: E402
from concourse.bass_utils import run_bass_kernel_spmd  # noqa: E402

B, T, N = 64, 1024, 1024
NCORES = 8
BL = B // NCORES  # 8 local batches per core
P = 128
KC = N // P  # 8 contraction chunks
KC8 = 4  # chunks 0..3 of the contraction run in fp8 (DoubleRow)
N8 = KC8 * P  # 512 fp8 contraction lanes
MT = N // P  # 8 output row tiles
F32 = mybir.dt.float32
BF16 = mybir.dt.bfloat16
FP8 = mybir.dt.float8e4
AF = mybir.ActivationFunctionType
ALU = mybir.AluOpType
DR = mybir.MatmulPerfMode.DoubleRow

LAST_EXEC_NS = None
_NC_CACHE = None


def build_bass():
    nc = bass.Bass()

    hT_h = nc.declare_dram_parameter("hT", [BL, N, T], BF16, isOutput=False)
    # fp8 chunks pre-paired on host: [b, pair, p, c, t], c = chunk within
    # pair -- one fully-contiguous DMA per pair, 2KB partition lines
    hT8_h = nc.declare_dram_parameter(
        "hT8", [BL, KC8 // 2, P, 2, T], FP8, isOutput=False
    )
    cov_h = nc.declare_dram_parameter("cov", [BL, T], F32, isOutput=False)
    covbc_h = nc.declare_dram_parameter("covbc", [BL, P, T], BF16, isOutput=False)
    mask_h = nc.declare_dram_parameter("mask", [BL, T], F32, isOutput=False)
    sT_h = nc.declare_dram_parameter("sT", [N, BL], BF16, isOutput=False)
    whT_h = nc.declare_dram_parameter("WhT", [N - N8, N], BF16, isOutput=False)
    whT8_h = nc.declare_dram_parameter(
        "WhT8", [KC8 // 2, P, 2, N], FP8, isOutput=False
    )
    # dec_W.T in scaled fp8 (x16 lifts values out of e4m3's subnormal
    # range; descaled by 1/16 at the dec_feaT eviction).  Halves the
    # startup-critical DMA prefix vs bf16; chunk pairs pre-interleaved for
    # 2KB partition lines.  Deterministic rel_err 1.871e-2 (sim-exact).
    dwT_h = nc.declare_dram_parameter(
        "decWT8", [KC // 2, P, 2, N], FP8, isOutput=False
    )
    decb_h = nc.declare_dram_parameter("decb", [1, N], BF16, isOutput=False)
    wcT_h = nc.declare_dram_parameter("WcT", [P, KC], F32, isOutput=False)
    maskcol_h = nc.declare_dram_parameter("maskcol", [P, KC], F32, isOutput=False)
    vcol_h = nc.declare_dram_parameter("vcol", [P, KC], BF16, isOutput=False)
    hnatl_h = nc.declare_dram_parameter("hnatl", [T, N], BF16, isOutput=False)

    ct_out = nc.declare_dram_parameter("out_ct", [BL, N], F32, isOutput=True)
    attn_out = nc.declare_dram_parameter("out_attn", [BL, T], F32, isOutput=True)
    cov_out = nc.declare_dram_parameter("out_cov", [BL, T], F32, isOutput=True)

    with tile.TileContext(nc) as tc:
        with (
            tc.tile_pool(name="const", bufs=1) as const,
            tc.tile_pool(name="ht", bufs=4) as htp,
            tc.tile_pool(name="ht8", bufs=4) as ht8p,
            tc.tile_pool(name="att", bufs=3) as attp,
            tc.tile_pool(name="rows", bufs=2) as rowp,
            tc.tile_pool(name="rows1", bufs=1) as rowp1,
            tc.tile_pool(name="bc", bufs=BL) as bcp,
            tc.tile_pool(name="scr", bufs=1) as scrp,
            tc.tile_pool(name="psA", bufs=2, space="PSUM") as psA,
            tc.tile_pool(name="psS", bufs=1, space="PSUM") as psS,
            tc.tile_pool(name="psB", bufs=1, space="PSUM") as psB,
        ):
            # ---- PE warm-up: dummy matmuls while the first DMAs land, so
            # the HAM clock gate reaches 2.4 GHz before real work starts ----
            ones_col = const.tile([1, P], BF16)  # also lhsT for broadcasts
            nc.any.memset(ones_col[:], 1.0)
            warm_row = const.tile([1, 512], BF16)
            nc.any.memset(warm_row[:], 0.0)
            onef = const.tile([1, 1], F32)  # permutation rhs for transposes
            nc.any.memset(onef[:], 1.0)
            # 10 is enough to cover the dwt DMA and start the clock ramp;
            # the first real matmuls finish ramping (a longer warm-up run
            # delays batch 0 past its operands' DMA arrival)
            ps_w = psA.tile([P, T], F32, tag="psA")
            for _ in range(10):
                nc.tensor.matmul(
                    ps_w[:, 0:512], ones_col[:], warm_row[:],
                    start=True, stop=True,
                )

            # ---- constants (issue order matters: prologue inputs first) ----
            # contraction split: chunks 0..KC8-1 in fp8 (DoubleRow), rest bf16
            wh8 = const.tile([P, KC8, N], FP8)  # [n%128, n//128, m], n < N8
            wh = const.tile([P, KC - KC8, N], BF16)  # n >= N8
            vcol = const.tile([P, KC], BF16)
            ct_all = const.tile([P, BL, KC], F32)  # c_t[p + 128*kc] of batch b
            wcT = const.tile([P, KC], F32)  # W_c[mt*128+p] per-partition scalars
            dec_feaT = const.tile([P, MT, BL], F32)  # dec_fea[m, b] bias layout
            # last batch gets a PE-based pass B (PE is idle at the tail):
            # h natural [t, n] tiles + attn as columns
            hnat_sb = const.tile([P, KC, N], BF16)
            atn_col = const.tile([P, KC], BF16)
            maskcol = const.tile([P, KC], F32)  # last batch's mask, columns
            nc.sync.dma_start(out=maskcol[:], in_=maskcol_h[:])

            # cov rows pre-broadcast across partitions on the host.  Tiles
            # allocated up front; DMAs sequenced into the startup order
            # below (batches 0-1) or one batch ahead of use (load_ht).
            cov_bc_all = []
            for b in range(BL):
                cb = bcp.tile([P, T], BF16, tag="covbc")
                cov_bc_all.append(cb)

            def load_ht_hi(b):
                # fp8 chunks + bf16 chunks KC8.. feed the main matmul
                # immediately; bf16 chunks 0..KC8-1 (pass-B-only) come last.
                t = htp.tile([P, KC, T], BF16, tag="ht")
                for kc in range(KC8, KC):
                    nc.sync.dma_start(
                        out=t[:, kc, :], in_=hT_h[b, kc * P : (kc + 1) * P, :]
                    )
                t8 = ht8p.tile([P, KC8, T], FP8, tag="ht8")
                for j in range(KC8 // 2):
                    # chunk pairs in one DMA: 2KB partition lines (a lone
                    # fp8 [128, T] chunk is only 1KB/partition, half rate)
                    nc.sync.dma_start(
                        out=t8[:, 2 * j : 2 * j + 2, :], in_=hT8_h[b, j]
                    )
                return t, t8

            def load_ht_lo(t, b):
                for kc in range(KC8):
                    nc.sync.dma_start(
                        out=t[:, kc, :], in_=hT_h[b, kc * P : (kc + 1) * P, :]
                    )

            def load_ht(b):
                t, t8 = load_ht_hi(b)
                # cov broadcast rows ride one batch ahead of their use
                if 1 <= b < BL - 1:
                    nc.sync.dma_start(
                        out=cov_bc_all[b + 1][:], in_=covbc_h[b + 1]
                    )
                load_ht_lo(t, b)
                return t, t8

            def load_rows(b):
                mrow = rowp.tile([1, T], F32, tag="mask")
                nc.sync.dma_start(out=mrow[:], in_=mask_h[b : b + 1, :])
                covrow = rowp.tile([1, T], F32, tag="covrow")
                nc.sync.dma_start(out=covrow[:], in_=cov_h[b : b + 1, :])
                return mrow, covrow

            # pass-B work is deferred and trickled into the next batch's
            # matmul loop so the DVE never bursts >1 op between PSUM
            # evictions (which would stall the PE on PSUM slot reuse).
            pending_pass_b = []

            def issue_pass_b_one():
                if pending_pass_b:
                    pending_pass_b.pop(0)()

            # ---- prologue: dec_fea = s_t_hat @ dec_W.T + dec_b  -> [b, m] ----
            with tc.tile_pool(name="prol", bufs=1) as prol:
                st = prol.tile([P, KC, BL], BF16, tag="st")
                nc.sync.dma_start(
                    out=st[:], in_=sT_h[:].rearrange("(kc p) b -> p kc b", p=P)
                )
                ones1 = prol.tile([1, BL], BF16, tag="ones1")
                nc.any.memset(ones1[:], 1.0)
                # batch-0's main-matmul operands outrank dec_W in DMA queue
                # order: mains can start ~6us earlier, and the dec matmuls
                # (one continuous clump when the 4 big dwt pair-DMAs land)
                # slot in between batch-0 groups without gapping the clock
                for j in range(KC8 // 2):
                    nc.sync.dma_start(
                        out=wh8[:, 2 * j : 2 * j + 2, :], in_=whT8_h[j]
                    )
                for kc in range(KC - KC8):
                    nc.sync.dma_start(
                        out=wh[:, kc, :], in_=whT_h[kc * P : (kc + 1) * P, :]
                    )
                ht0, ht0_8 = load_ht_hi(0)
                nc.sync.dma_start(out=cov_bc_all[0][:], in_=covbc_h[0])
                nc.sync.dma_start(out=wcT[:], in_=wcT_h[:])
                nc.sync.dma_start(out=vcol[:], in_=vcol_h[:])
                db = prol.tile([1, N], BF16, tag="db")
                nc.sync.dma_start(out=db[:], in_=decb_h[:])
                dwt = prol.tile([P, KC, N], FP8, tag="dwt")
                for j in range(KC // 2):
                    nc.sync.dma_start(
                        out=dwt[:, 2 * j : 2 * j + 2, :], in_=dwT_h[j]
                    )
                nc.sync.dma_start(out=cov_bc_all[1][:], in_=covbc_h[1])
                # dec_feaT[m, b] = sum_n dec_W[m, n] s_t_hat[b, n] + dec_b[m]
                for mt in range(MT):
                    msl = slice(mt * P, (mt + 1) * P)
                    ps_d = psA.tile([P, BL], F32, tag="psA")
                    for kc in range(KC):
                        nc.tensor.matmul(
                            ps_d[:, :],
                            dwt[:, kc, msl],
                            st[:, kc, :],
                            start=(kc == 0),
                            stop=False,
                        )
                    nc.tensor.matmul(
                        ps_d[:, :], db[:, msl], ones1[:],
                        start=False, stop=True,
                    )
                    # Scalar evicts (DVE is busier during batch 0) and
                    # descales the x16 fp8 dec_W domain back to natural units
                    nc.scalar.mul(dec_feaT[:, mt, :], ps_d[:, :], 1.0 / 16.0)

            load_ht_lo(ht0, 0)
            ht_next = (ht0, ht0_8)
            rows_next = load_rows(0)
            # second h batch in flight too: two-deep prefetch absorbs DMA
            # service-order jitter for the whole pipeline (rows are tiny and
            # stay one-deep)
            ht_next2 = load_ht(1)
            for b in range(BL):
                ht, ht8 = ht_next
                mrow, covrow = rows_next

                cov_bc = cov_bc_all[b]
                ps_sc = psS.tile([1, T], F32, tag="psS")
                for mt in range(MT):
                    msl = slice(mt * P, (mt + 1) * P)
                    ps_att = psA.tile([P, T], F32, tag="psA")
                    for th in range(2):
                        sl = slice(th * 512, (th + 1) * 512)
                        # bf16 chunks first (cheap 128-row LoadStationary on
                        # the group-opening slot), fp8 DoubleRow K=256 last
                        for kc in range(KC - KC8):
                            nc.tensor.matmul(
                                ps_att[:, sl],
                                wh[:, kc, msl],
                                ht[:, kc + KC8, sl],
                                start=(kc == 0),
                                stop=False,
                            )
                        for j in range(KC8 // 2):
                            nc.tensor.matmul(
                                ps_att[:, sl],
                                wh8[:, 2 * j : 2 * j + 2, msl],
                                ht8[:, 2 * j : 2 * j + 2, sl],
                                start=False,
                                stop=(j == KC8 // 2 - 1),
                                perf_mode=DR,
                            )
                    # att += W_c[m] * cov[t]  (fused on DVE, in place on PSUM;
                    # GpSimd cannot access PSUM)
                    nc.vector.scalar_tensor_tensor(
                        out=ps_att[:, :], in0=cov_bc[:, :],
                        scalar=wcT[:, mt : mt + 1], in1=ps_att[:, :],
                        op0=ALU.mult, op1=ALU.add,
                    )
                    att = attp.tile([P, T], BF16, tag="att")
                    # att = tanh(psum + dec_fea[m])  (bias folds the dec term)
                    nc.scalar.activation(
                        att[:], ps_att[:], AF.Tanh,
                        bias=dec_feaT[:, mt, b : b + 1],
                    )
                    if b == BL - 1 and mt == MT - 2:
                        att_m6 = att
                    if b == BL - 1 and mt == MT - 1:
                        att_tail = att
                    for th in range(2):
                        sl = slice(th * 512, (th + 1) * 512)
                        nc.tensor.matmul(
                            ps_sc[:, sl],
                            vcol[:, mt : mt + 1],
                            att[:, sl],
                            start=(mt == 0),
                            stop=(mt == MT - 1),
                        )
                    # during the last batch, keep b-2's pass-B DVE ops out of
                    # the stream so the final softmax chain isn't queued
                    # behind ~10us of STTs; they drain at the end instead.
                    if b < BL - 1:
                        issue_pass_b_one()

                # prefetch next batch while this batch's softmax runs
                if b + 1 < BL:
                    ht_next = ht_next2
                    rows_next = load_rows(b + 1)
                if b + 2 < BL:
                    ht_next2 = load_ht(b + 2)
                if b == 1:
                    # h natural tiles for the last batch's PE pass B; loaded
                    # early, in a DMA window with slack
                    for tc_ in range(KC):
                        nc.sync.dma_start(
                            out=hnat_sb[:, tc_, :],
                            in_=hnatl_h[tc_ * P : (tc_ + 1) * P, :],
                        )

                # softmax over t (no max-subtraction: |score| <= ||v||_1 ~ 26)
                erow = rowp1.tile([1, T], F32, tag="erow")
                nc.scalar.activation(erow[:], ps_sc[:], AF.Exp)
                emrow = rowp1.tile([1, T], F32, tag="emrow")
                ssum = rowp1.tile([1, 1], F32, tag="ssum")
                nc.vector.scalar_tensor_tensor(
                    out=emrow[:], in0=erow[:], scalar=1.0, in1=mrow[:],
                    op0=ALU.bypass, op1=ALU.mult, accum_out=ssum[:],
                )
                rinv = rowp1.tile([1, 1], F32, tag="rinv")
                nc.vector.reciprocal(rinv[:], ssum[:])
                arow = rowp.tile([1, T], F32, tag="arow")
                nc.vector.tensor_scalar_mul(arow[:], emrow[:], rinv[:])
                nc.sync.dma_start(out=attn_out[b : b + 1, :], in_=arow[:])
                cnrow = rowp1.tile([1, T], F32, tag="cnrow")
                nc.vector.tensor_add(cnrow[:], arow[:], covrow[:])
                nc.sync.dma_start(out=cov_out[b : b + 1, :], in_=cnrow[:])

                # pass B: c_t[n] = sum_t attn[t] * hT[n, t]
                if b < BL - 1:
                    # bf16 attn row for the DVE pass B (unused by the last
                    # batch, which goes through the PE transpose path)
                    abrow = rowp.tile([1, T], BF16, tag="abrow")
                    nc.vector.tensor_copy(abrow[:], arow[:])
                else:
                    abrow = None

                def make_pass_b(ht_=ht, b_=b, abrow_=abrow, erow_=erow,
                                rinv_=rinv):
                    ps_box = []

                    def atn_tp():
                        # last batch works on the unnormalized exp row (the
                        # mask folds into the columnar copy below, the 1/sum
                        # into the final psum eviction): PE transposes turn
                        # it into [128, 8] columns without waiting on the
                        # emrow STT or a DRAM-bounce round trip
                        tp_ps = psB.tile([P, KC], F32, tag="psB")
                        for c in range(KC):
                            nc.tensor.matmul(
                                tp_ps[:, c : c + 1],
                                erow_[0:1, c * P : (c + 1) * P],
                                onef[:],
                                start=(c == 0), stop=(c == KC - 1),
                                is_transpose=True,
                            )
                        nc.vector.tensor_mul(atn_col[:], tp_ps[:], maskcol[:])

                    def pe_ct():
                        ps_fin = psS.tile([1, N], F32, tag="psS")
                        ps_box.append(ps_fin)
                        for th in range(2):
                            sl = slice(th * 512, (th + 1) * 512)
                            for tc in range(KC):
                                nc.tensor.matmul(
                                    ps_fin[0:1, sl],
                                    atn_col[:, tc : tc + 1],
                                    hnat_sb[:, tc, sl],
                                    start=(tc == 0),
                                    stop=(tc == KC - 1),
                                )

                    def ct_evict():
                        ctrow = rowp1.tile([1, N], F32, tag="ctrow")
                        nc.vector.tensor_scalar_mul(
                            ctrow[:], ps_box[0][:], rinv_[:]
                        )
                        nc.sync.dma_start(
                            out=ct_out[b_ : b_ + 1, :], in_=ctrow[:]
                        )

                    def bcast():
                        ps_bc = psB.tile([P, T], F32, tag="psB")
                        ps_box.append(ps_bc)
                        for th in range(2):
                            sl = slice(th * 512, (th + 1) * 512)
                            nc.tensor.matmul(
                                ps_bc[:, sl], ones_col[:], abrow_[:, sl],
                                start=True, stop=True,
                            )

                    def stt_one(kc):
                        def run():
                            sc = scrp.tile([P, T], BF16, tag="scr")
                            nc.vector.scalar_tensor_tensor(
                                out=sc[:], in0=ht_[:, kc, :], scalar=1.0,
                                in1=ps_box[0][:], op0=ALU.bypass, op1=ALU.mult,
                                accum_out=ct_all[:, b_, kc : kc + 1],
                            )
                        return run

                    def ct_dma():
                        # c_t[b] out: dest viewed [p, kc] (4B-strided, tiny)
                        nc.sync.dma_start(
                            out=ct_out[b_ : b_ + 1, :].rearrange(
                                "o (k p) -> (o p) k", p=P
                            ),
                            in_=ct_all[:, b_, :],
                        )

                    if b_ == BL - 1:
                        return [atn_tp, pe_ct, ct_evict]
                    return (
                        [bcast]
                        + [stt_one(kc) for kc in range(KC)]
                        + [ct_dma]
                    )

                if b == BL - 1:
                    # issue the last batch's tail chain FIRST (the engine
                    # streams are drained ready-first in issue order): its
                    # pass B must not queue behind b-1's leftover DVE work
                    for fn in make_pass_b():
                        fn()
                else:
                    pending_pass_b.extend(make_pass_b())
                    issue_pass_b_one()
                    issue_pass_b_one()

            # keep the PE clock warm through the last batch's eviction and
            # exp waits.  Reading the mt6/mt7 att tiles (ready right after
            # their tanh) pins these into exactly those windows --
            # dependency-free warmups would be hoisted to any earlier idle
            # PE slot by the scheduler.
            ps_tw = psA.tile([P, T], F32, tag="psA")
            for _ in range(5):
                nc.tensor.matmul(
                    ps_tw[:, 0:512], ones_col[:], att_m6[0:1, 0:512],
                    start=True, stop=True,
                )
            for _ in range(4):
                nc.tensor.matmul(
                    ps_tw[:, 0:512], ones_col[:], att_tail[0:1, 0:512],
                    start=True, stop=True,
                )

            while pending_pass_b:
                issue_pass_b_one()

    _legalize_waits(nc)
    return nc


# Walrus rejects instructions whose sync-wait count exceeds the per-opcode
# descriptor slots ("Too many sync wait commands").  Tile can emit 2-3 waits
# on matmuls/DMAs at cross-engine convergence points.  Hoist surplus waits
# onto standalone InstEventSemaphore carriers inserted just before the
# offender in the same engine stream: the engine stalls on the carrier(s),
# then issues the real instruction with a single wait.  Engine streams are
# in-order, so this is semantics-preserving.
_WAIT_SKIP_OPS = {"InstEventSemaphore"}


def _legalize_waits(nc, limit=1):
    import bass_rust

    def make_carrier(engine, wait):
        return mybir.InstNoOp(
            name=nc.get_next_instruction_name(),
            text_hint="waitfix",
            bass_nofuse=True,
            engine=engine,
            sync_info=mybir.SyncInfo(on_wait=[wait], on_update=[]),
        )

    for fn in nc.m.functions:
        for blk in fn.blocks:
            il = blk.instructions
            i = 0
            while i < len(il):
                inst = il[i]
                op = type(inst).__name__
                si = getattr(inst, "sync_info", None)
                if (
                    op in _WAIT_SKIP_OPS
                    or si is None
                    or len(si.on_wait) <= limit
                ):
                    i += 1
                    continue
                waits = list(si.on_wait)
                keep, surplus = waits[-limit:], waits[:-limit]
                carriers = [make_carrier(inst.engine, w) for w in surplus]
                inst.sync_info = bass_rust.SyncInfo(
                    on_wait=keep, on_update=si.on_update
                )
                for k, ev in enumerate(carriers):
                    il.insert(i + k, ev)
                i += len(carriers) + 1


def _get_nc():
    global _NC_CACHE
    if _NC_CACHE is None:
        _NC_CACHE = build_bass()
    return _NC_CACHE


def kernel(s_t_hat, h, enc_padding_mask, coverage, W_h, W_c, dec_W, dec_b, v):
    global LAST_EXEC_NS
    import ml_dtypes

    bf16 = ml_dtypes.bfloat16
    fp8 = ml_dtypes.float8_e4m3
    s_t_hat = np.asarray(s_t_hat, dtype=np.float32)
    h = np.asarray(h, dtype=np.float32)
    enc_padding_mask = np.ascontiguousarray(
        np.asarray(enc_padding_mask, dtype=np.float32)
    )
    coverage = np.ascontiguousarray(np.asarray(coverage, dtype=np.float32))
    W_h = np.asarray(W_h, dtype=np.float32)
    W_c = np.asarray(W_c, dtype=np.float32).reshape(1, N)
    dec_W = np.asarray(dec_W, dtype=np.float32)
    dec_b = np.asarray(dec_b, dtype=np.float32).reshape(1, N)
    v = np.asarray(v, dtype=np.float32)

    hTf = np.transpose(h, (0, 2, 1))  # [B, N, T] fp32 view
    hT = np.ascontiguousarray(hTf.astype(bf16))  # [B, N, T]
    # fp8 chunk pairs: [b, pair, p, c, t] with n = (2*pair + c)*128 + p
    hT8 = np.ascontiguousarray(
        hTf[:, :N8, :].astype(fp8)
        .reshape(B, KC8 // 2, 2, P, T)
        .transpose(0, 1, 3, 2, 4)
    )
    WhTf = W_h.T  # [n, m] fp32
    WhT = np.ascontiguousarray(WhTf[N8:, :].astype(bf16))  # bf16 tail chunks
    WhT8 = np.ascontiguousarray(
        WhTf[:N8, :].astype(fp8)
        .reshape(KC8 // 2, 2, P, N)
        .transpose(0, 2, 1, 3)
    )
    # dec_W.T x16 in fp8, chunk pairs interleaved: [pair, p, c, m] with
    # n = (2*pair + c)*128 + p.  The x16 scale is undone on-device at the
    # dec_feaT eviction; dec_b ships pre-scaled to match.
    decWT8 = np.ascontiguousarray(
        (dec_W.T.astype(np.float32) * 16.0).astype(fp8)
        .reshape(KC // 2, 2, P, N)
        .transpose(0, 2, 1, 3)
    )
    sT = np.ascontiguousarray(s_t_hat.T.astype(bf16))  # [n, B]
    vcol = np.ascontiguousarray(v.reshape(KC, P).T.astype(bf16))  # [p, kc]
    covbc = np.ascontiguousarray(
        np.broadcast_to(
            coverage.astype(bf16)[:, None, :], (B, P, T)
        )
    )  # [B, p, T] cov rows replicated across partitions
    wcT = np.ascontiguousarray(
        W_c.reshape(KC, P).T.astype(np.float32)
    )  # [p, kc]
    decb_b = np.ascontiguousarray((dec_b.astype(np.float32) * 16.0).astype(bf16))

    in_maps = []
    for c in range(NCORES):
        bs = slice(c * BL, (c + 1) * BL)
        in_maps.append(
            {
                "hT": hT[bs],
                "hT8": hT8[bs],
                "hnatl": np.ascontiguousarray(
                    h[(c + 1) * BL - 1].astype(bf16)
                ),
                "maskcol": np.ascontiguousarray(
                    enc_padding_mask[(c + 1) * BL - 1]
                    .reshape(KC, P).T.astype(np.float32)
                ),
                "cov": coverage[bs],
                "covbc": covbc[bs],
                "mask": enc_padding_mask[bs],
                "sT": np.ascontiguousarray(sT[:, bs]),
                "WhT": WhT,
                "WhT8": WhT8,
                "decWT8": decWT8,
                "decb": decb_b,
                "WcT": wcT,
                "vcol": vcol,
            }
        )

    nc = _get_nc()
    trace = os.environ.get("BASS_KERNEL_TRACE", "0") == "1"
    res = run_bass_kernel_spmd(
        nc, in_maps, core_ids=list(range(NCORES)), trace=trace
    )
    LAST_EXEC_NS = res.exec_time_ns

    c_t = np.concatenate([res.results[c]["out_ct"] for c in range(NCORES)], axis=0)
    attn = np.concatenate(
        [res.results[c]["out_attn"] for c in range(NCORES)], axis=0
    )
    cov_new = np.concatenate(
        [res.results[c]["out_cov"] for c in range(NCORES)], axis=0
    )
    return (c_t, attn, cov_new)



# revision 84
# speedup vs baseline: 1.0243x; 1.0243x over previous
"""Pointer-generator attention kernel for 8 TRN2 NeuronCores.

Computation (per batch b):
    enc_feat = h[b] @ W_h.T                       # [T, N]
    att      = enc_feat + dec_fea[b] + cov[b,:,None] * W_c
    scores   = tanh(att) @ v                      # [T]
    attn     = exp(scores) * mask / sum(...)      # [T]
    c_t      = attn @ h[b]                        # [N]
    cov_new  = cov + attn

Sharding: data-parallel over batch, 8 batches per core, no collectives.

Device-side layout (per core):
    hT [8, N, T] -- h transposed per batch, so the contraction dim n sits
    on SBUF partitions for the main matmul AND the t axis is the free dim
    for the pass-B reduce.
    Main matmul precision is a hybrid K-split: contraction chunks 0..3 run
    as fp8e4 DoubleRow matmuls (both operands fp8, K=256 per instruction,
    2x PE throughput), chunks 4..7 in bf16; fp32 PSUM accumulation.  The
    split is accuracy-bound: measured max rel err 1.65e-2 of the 2e-2
    budget (pure fp8 would be 2.27e-2).  h ships twice: fp8 chunks 0..3
    (matmul) + full bf16 (bf16 matmul chunks + the pass-B reduce), with
    the pass-B-only chunks at lowest DMA priority and h prefetched two
    batches deep.
    att tiles [m=128, t=1024]: lhsT = W_hT chunk (stationary), rhs = hT.
    cov term via DVE STT in place on PSUM (cov rows pre-broadcast across
    partitions on the host), dec_fea folded into the tanh bias on ScalarE,
    v-dot as M=1 matmuls on PE, softmax on single-partition rows (exp has
    no overflow risk: |score| <= ||v||_1 ~ 26), pass B as fused
    multiply+reduce on VectorE over the resident hT tiles.
    Last batch's pass B runs on the PE instead (DVE would backlog the
    final softmax): the unnormalized exp row is transposed into [128, 8]
    columns by PE transpose matmuls (mask folded in the columnar copy,
    1/sum folded into the psum eviction) and contracted against natural-
    layout h tiles; warm-up matmuls pinned to the last att tiles keep the
    PE clock hot through that window.
"""

import os
import sys

import numpy as np

sys.path.insert(0, "/opt/trn_rl_repo")

import concourse.bass as bass  # noqa: E402
import concourse.tile as tile  # noqa: E402
from concourse import mybir  # noqa: E402
from concourse.bass_utils import run_bass_kernel_spmd  # noqa: E402

B, T, N = 64, 1024, 1024
NCORES = 8
BL = B // NCORES  # 8 local batches per core
P = 128
KC = N // P  # 8 contraction chunks
KC8 = 4  # chunks 0..3 of the contraction run in fp8 (DoubleRow)
N8 = KC8 * P  # 512 fp8 contraction lanes
MT = N // P  # 8 output row tiles
F32 = mybir.dt.float32
BF16 = mybir.dt.bfloat16
FP8 = mybir.dt.float8e4
AF = mybir.ActivationFunctionType
ALU = mybir.AluOpType
DR = mybir.MatmulPerfMode.DoubleRow

LAST_EXEC_NS = None
_NC_CACHE = None


def build_bass():
    nc = bass.Bass()

    hT_h = nc.declare_dram_parameter("hT", [BL, N, T], BF16, isOutput=False)
    # fp8 chunks pre-paired on host: [b, pair, p, c, t], c = chunk within
    # pair -- one fully-contiguous DMA per pair, 2KB partition lines
    hT8_h = nc.declare_dram_parameter(
        "hT8", [BL, KC8 // 2, P, 2, T], FP8, isOutput=False
    )
    cov_h = nc.declare_dram_parameter("cov", [BL, T], F32, isOutput=False)
    covbc_h = nc.declare_dram_parameter("covbc", [BL, P, T], BF16, isOutput=False)
    mask_h = nc.declare_dram_parameter("mask", [BL, T], F32, isOutput=False)
    sT_h = nc.declare_dram_parameter("sT", [N, BL], BF16, isOutput=False)
    whT_h = nc.declare_dram_parameter("WhT", [N - N8, N], BF16, isOutput=False)
    whT8_h = nc.declare_dram_parameter(
        "WhT8", [KC8 // 2, P, 2, N], FP8, isOutput=False
    )
    # dec_W.T in scaled fp8 (x16 lifts values out of e4m3's subnormal
    # range; descaled by 1/16 at the dec_feaT eviction).  Halves the
    # startup-critical DMA prefix vs bf16; chunk pairs pre-interleaved for
    # 2KB partition lines.  Deterministic rel_err 1.871e-2 (sim-exact).
    dwT_h = nc.declare_dram_parameter(
        "decWT8", [KC // 2, P, 2, N], FP8, isOutput=False
    )
    decb_h = nc.declare_dram_parameter("decb", [1, N], BF16, isOutput=False)
    wcT_h = nc.declare_dram_parameter("WcT", [P, KC], F32, isOutput=False)
    maskcol_h = nc.declare_dram_parameter("maskcol", [P, KC], F32, isOutput=False)
    vcol_h = nc.declare_dram_parameter("vcol", [P, KC], BF16, isOutput=False)
    hnatl_h = nc.declare_dram_parameter("hnatl", [T, N], BF16, isOutput=False)

    ct_out = nc.declare_dram_parameter("out_ct", [BL, N], F32, isOutput=True)
    attn_out = nc.declare_dram_parameter("out_attn", [BL, T], F32, isOutput=True)
    cov_out = nc.declare_dram_parameter("out_cov", [BL, T], F32, isOutput=True)

    with tile.TileContext(nc) as tc:
        with (
            tc.tile_pool(name="const", bufs=1) as const,
            tc.tile_pool(name="ht", bufs=4) as htp,
            tc.tile_pool(name="ht8", bufs=4) as ht8p,
            tc.tile_pool(name="att", bufs=3) as attp,
            tc.tile_pool(name="rows", bufs=2) as rowp,
            tc.tile_pool(name="rows1", bufs=1) as rowp1,
            tc.tile_pool(name="bc", bufs=BL) as bcp,
            tc.tile_pool(name="scr", bufs=1) as scrp,
            tc.tile_pool(name="psA", bufs=2, space="PSUM") as psA,
            tc.tile_pool(name="psS", bufs=1, space="PSUM") as psS,
            tc.tile_pool(name="psB", bufs=1, space="PSUM") as psB,
        ):
            # ---- PE warm-up: dummy matmuls while the first DMAs land, so
            # the HAM clock gate reaches 2.4 GHz before real work starts ----
            ones_col = const.tile([1, P], BF16)  # also lhsT for broadcasts
            nc.any.memset(ones_col[:], 1.0)
            warm_row = const.tile([1, 512], BF16)
            nc.any.memset(warm_row[:], 0.0)
            onef = const.tile([1, 1], F32)  # permutation rhs for transposes
            nc.any.memset(onef[:], 1.0)
            # 10 is enough to cover the dwt DMA and start the clock ramp;
            # the first real matmuls finish ramping (a longer warm-up run
            # delays batch 0 past its operands' DMA arrival)
            ps_w = psA.tile([P, T], F32, tag="psA")
            for _ in range(10):
                nc.tensor.matmul(
                    ps_w[:, 0:512], ones_col[:], warm_row[:],
                    start=True, stop=True,
                )

            # ---- constants (issue order matters: prologue inputs first) ----
            # contraction split: chunks 0..KC8-1 in fp8 (DoubleRow), rest bf16
            wh8 = const.tile([P, KC8, N], FP8)  # [n%128, n//128, m], n < N8
            wh = const.tile([P, KC - KC8, N], BF16)  # n >= N8
            vcol = const.tile([P, KC], BF16)
            ct_all = const.tile([P, BL, KC], F32)  # c_t[p + 128*kc] of batch b
            wcT = const.tile([P, KC], F32)  # W_c[mt*128+p] per-partition scalars
            dec_feaT = const.tile([P, MT, BL], F32)  # dec_fea[m, b] bias layout
            # last batch gets a PE-based pass B (PE is idle at the tail):
            # h natural [t, n] tiles + attn as columns
            hnat_sb = const.tile([P, KC, N], BF16)
            atn_col = const.tile([P, KC], BF16)
            maskcol = const.tile([P, KC], F32)  # last batch's mask, columns
            nc.sync.dma_start(out=maskcol[:], in_=maskcol_h[:])

            # cov rows pre-broadcast across partitions on the host.  Tiles
            # allocated up front; DMAs sequenced into the startup order
            # below (batches 0-1) or one batch ahead of use (load_ht).
            cov_bc_all = []
            for b in range(BL):
                cb = bcp.tile([P, T], BF16, tag="covbc")
                cov_bc_all.append(cb)

            def load_ht_hi(b):
                # fp8 chunks + bf16 chunks KC8.. feed the main matmul
                # immediately; bf16 chunks 0..KC8-1 (pass-B-only) come last.
                t = htp.tile([P, KC, T], BF16, tag="ht")
                for kc in range(KC8, KC):
                    nc.sync.dma_start(
                        out=t[:, kc, :], in_=hT_h[b, kc * P : (kc + 1) * P, :]
                    )
                t8 = ht8p.tile([P, KC8, T], FP8, tag="ht8")
                for j in range(KC8 // 2):
                    # chunk pairs in one DMA: 2KB partition lines (a lone
                    # fp8 [128, T] chunk is only 1KB/partition, half rate)
                    nc.sync.dma_start(
                        out=t8[:, 2 * j : 2 * j + 2, :], in_=hT8_h[b, j]
                    )
                return t, t8

            def load_ht_lo(t, b):
                for kc in range(KC8):
                    nc.sync.dma_start(
                        out=t[:, kc, :], in_=hT_h[b, kc * P : (kc + 1) * P, :]
                    )

            def load_ht(b):
                t, t8 = load_ht_hi(b)
                # cov broadcast rows ride one batch ahead of their use
                if 1 <= b < BL - 1:
                    nc.sync.dma_start(
                        out=cov_bc_all[b + 1][:], in_=covbc_h[b + 1]
                    )
                load_ht_lo(t, b)
                return t, t8

            def load_rows(b):
                mrow = rowp.tile([1, T], F32, tag="mask")
                nc.sync.dma_start(out=mrow[:], in_=mask_h[b : b + 1, :])
                covrow = rowp.tile([1, T], F32, tag="covrow")
                nc.sync.dma_start(out=covrow[:], in_=cov_h[b : b + 1, :])
                return mrow, covrow

            # pass-B work is deferred and trickled into the next batch's
            # matmul loop so the DVE never bursts >1 op between PSUM
            # evictions (which would stall the PE on PSUM slot reuse).
            pending_pass_b = []

            def issue_pass_b_one():
                if pending_pass_b:
                    pending_pass_b.pop(0)()

            # ---- prologue: dec_fea = s_t_hat @ dec_W.T + dec_b  -> [b, m] ----
            with tc.tile_pool(name="prol", bufs=1) as prol:
                st = prol.tile([P, KC, BL], BF16, tag="st")
                nc.sync.dma_start(
                    out=st[:], in_=sT_h[:].rearrange("(kc p) b -> p kc b", p=P)
                )
                ones1 = prol.tile([1, BL], BF16, tag="ones1")
                nc.any.memset(ones1[:], 1.0)
                db = prol.tile([1, N], BF16, tag="db")
                nc.sync.dma_start(out=db[:], in_=decb_h[:])
                dwt = prol.tile([P, KC, N], FP8, tag="dwt")
                for j in range(KC // 2):
                    nc.sync.dma_start(
                        out=dwt[:, 2 * j : 2 * j + 2, :], in_=dwT_h[j]
                    )
                for j in range(KC8 // 2):
                    nc.sync.dma_start(
                        out=wh8[:, 2 * j : 2 * j + 2, :], in_=whT8_h[j]
                    )
                for kc in range(KC - KC8):
                    nc.sync.dma_start(
                        out=wh[:, kc, :], in_=whT_h[kc * P : (kc + 1) * P, :]
                    )
                nc.sync.dma_start(out=vcol[:], in_=vcol_h[:])
                nc.sync.dma_start(out=wcT[:], in_=wcT_h[:])
                # dec_feaT[m, b] = sum_n dec_W[m, n] s_t_hat[b, n] + dec_b[m]
                for mt in range(MT):
                    msl = slice(mt * P, (mt + 1) * P)
                    ps_d = psA.tile([P, BL], F32, tag="psA")
                    for kc in range(KC):
                        nc.tensor.matmul(
                            ps_d[:, :],
                            dwt[:, kc, msl],
                            st[:, kc, :],
                            start=(kc == 0),
                            stop=False,
                        )
                    nc.tensor.matmul(
                        ps_d[:, :], db[:, msl], ones1[:],
                        start=False, stop=True,
                    )
                    # Scalar evicts (DVE is busier during batch 0) and
                    # descales the x16 fp8 dec_W domain back to natural units
                    nc.scalar.mul(dec_feaT[:, mt, :], ps_d[:, :], 1.0 / 16.0)

            ht0, ht0_8 = load_ht_hi(0)
            nc.sync.dma_start(out=cov_bc_all[0][:], in_=covbc_h[0])
            nc.sync.dma_start(out=cov_bc_all[1][:], in_=covbc_h[1])
            load_ht_lo(ht0, 0)
            ht_next = (ht0, ht0_8)
            rows_next = load_rows(0)
            # second h batch in flight too: two-deep prefetch absorbs DMA
            # service-order jitter for the whole pipeline (rows are tiny and
            # stay one-deep)
            ht_next2 = load_ht(1)
            for b in range(BL):
                ht, ht8 = ht_next
                mrow, covrow = rows_next

                cov_bc = cov_bc_all[b]
                ps_sc = psS.tile([1, T], F32, tag="psS")
                for mt in range(MT):
                    msl = slice(mt * P, (mt + 1) * P)
                    ps_att = psA.tile([P, T], F32, tag="psA")
                    for th in range(2):
                        sl = slice(th * 512, (th + 1) * 512)
                        # bf16 chunks first (cheap 128-row LoadStationary on
                        # the group-opening slot), fp8 DoubleRow K=256 last
                        for kc in range(KC - KC8):
                            nc.tensor.matmul(
                                ps_att[:, sl],
                                wh[:, kc, msl],
                                ht[:, kc + KC8, sl],
                                start=(kc == 0),
                                stop=False,
                            )
                        for j in range(KC8 // 2):
                            nc.tensor.matmul(
                                ps_att[:, sl],
                                wh8[:, 2 * j : 2 * j + 2, msl],
                                ht8[:, 2 * j : 2 * j + 2, sl],
                                start=False,
                                stop=(j == KC8 // 2 - 1),
                                perf_mode=DR,
                            )
                    # att += W_c[m] * cov[t]  (fused on DVE, in place on PSUM;
                    # GpSimd cannot access PSUM)
                    nc.vector.scalar_tensor_tensor(
                        out=ps_att[:, :], in0=cov_bc[:, :],
                        scalar=wcT[:, mt : mt + 1], in1=ps_att[:, :],
                        op0=ALU.mult, op1=ALU.add,
                    )
                    att = attp.tile([P, T], BF16, tag="att")
                    # att = tanh(psum + dec_fea[m])  (bias folds the dec term)
                    nc.scalar.activation(
                        att[:], ps_att[:], AF.Tanh,
                        bias=dec_feaT[:, mt, b : b + 1],
                    )
                    if b == BL - 1 and mt == MT - 2:
                        att_m6 = att
                    if b == BL - 1 and mt == MT - 1:
                        att_tail = att
                    for th in range(2):
                        sl = slice(th * 512, (th + 1) * 512)
                        nc.tensor.matmul(
                            ps_sc[:, sl],
                            vcol[:, mt : mt + 1],
                            att[:, sl],
                            start=(mt == 0),
                            stop=(mt == MT - 1),
                        )
                    # during the last batch, keep b-2's pass-B DVE ops out of
                    # the stream so the final softmax chain isn't queued
                    # behind ~10us of STTs; they drain at the end instead.
                    if b < BL - 1:
                        issue_pass_b_one()

                # prefetch next batch while this batch's softmax runs
                if b + 1 < BL:
                    ht_next = ht_next2
                    rows_next = load_rows(b + 1)
                if b + 2 < BL:
                    ht_next2 = load_ht(b + 2)
                if b == 1:
                    # h natural tiles for the last batch's PE pass B; loaded
                    # early, in a DMA window with slack
                    for tc_ in range(KC):
                        nc.sync.dma_start(
                            out=hnat_sb[:, tc_, :],
                            in_=hnatl_h[tc_ * P : (tc_ + 1) * P, :],
                        )

                # softmax over t (no max-subtraction: |score| <= ||v||_1 ~ 26)
                erow = rowp1.tile([1, T], F32, tag="erow")
                nc.scalar.activation(erow[:], ps_sc[:], AF.Exp)
                emrow = rowp1.tile([1, T], F32, tag="emrow")
                ssum = rowp1.tile([1, 1], F32, tag="ssum")
                nc.vector.scalar_tensor_tensor(
                    out=emrow[:], in0=erow[:], scalar=1.0, in1=mrow[:],
                    op0=ALU.bypass, op1=ALU.mult, accum_out=ssum[:],
                )
                rinv = rowp1.tile([1, 1], F32, tag="rinv")
                nc.vector.reciprocal(rinv[:], ssum[:])
                arow = rowp.tile([1, T], F32, tag="arow")
                nc.vector.tensor_scalar_mul(arow[:], emrow[:], rinv[:])
                nc.sync.dma_start(out=attn_out[b : b + 1, :], in_=arow[:])
                cnrow = rowp1.tile([1, T], F32, tag="cnrow")
                nc.vector.tensor_add(cnrow[:], arow[:], covrow[:])
                nc.sync.dma_start(out=cov_out[b : b + 1, :], in_=cnrow[:])

                # pass B: c_t[n] = sum_t attn[t] * hT[n, t]
                if b < BL - 1:
                    # bf16 attn row for the DVE pass B (unused by the last
                    # batch, which goes through the PE transpose path)
                    abrow = rowp.tile([1, T], BF16, tag="abrow")
                    nc.vector.tensor_copy(abrow[:], arow[:])
                else:
                    abrow = None

                def make_pass_b(ht_=ht, b_=b, abrow_=abrow, erow_=erow,
                                rinv_=rinv):
                    ps_box = []

                    def atn_tp():
                        # last batch works on the unnormalized exp row (the
                        # mask folds into the columnar copy below, the 1/sum
                        # into the final psum eviction): PE transposes turn
                        # it into [128, 8] columns without waiting on the
                        # emrow STT or a DRAM-bounce round trip
                        tp_ps = psB.tile([P, KC], F32, tag="psB")
                        for c in range(KC):
                            nc.tensor.matmul(
                                tp_ps[:, c : c + 1],
                                erow_[0:1, c * P : (c + 1) * P],
                                onef[:],
                                start=(c == 0), stop=(c == KC - 1),
                                is_transpose=True,
                            )
                        nc.vector.tensor_mul(atn_col[:], tp_ps[:], maskcol[:])

                    def pe_ct():
                        ps_fin = psS.tile([1, N], F32, tag="psS")
                        ps_box.append(ps_fin)
                        for th in range(2):
                            sl = slice(th * 512, (th + 1) * 512)
                            for tc in range(KC):
                                nc.tensor.matmul(
                                    ps_fin[0:1, sl],
                                    atn_col[:, tc : tc + 1],
                                    hnat_sb[:, tc, sl],
                                    start=(tc == 0),
                                    stop=(tc == KC - 1),
                                )

                    def ct_evict():
                        ctrow = rowp1.tile([1, N], F32, tag="ctrow")
                        nc.vector.tensor_scalar_mul(
                            ctrow[:], ps_box[0][:], rinv_[:]
                        )
                        nc.sync.dma_start(
                            out=ct_out[b_ : b_ + 1, :], in_=ctrow[:]
                        )

                    def bcast():
                        ps_bc = psB.tile([P, T], F32, tag="psB")
                        ps_box.append(ps_bc)
                        for th in range(2):
                            sl = slice(th * 512, (th + 1) * 512)
                            nc.tensor.matmul(
                                ps_bc[:, sl], ones_col[:], abrow_[:, sl],
                                start=True, stop=True,
                            )

                    def stt_one(kc):
                        def run():
                            sc = scrp.tile([P, T], BF16, tag="scr")
                            nc.vector.scalar_tensor_tensor(
                                out=sc[:], in0=ht_[:, kc, :], scalar=1.0,
                                in1=ps_box[0][:], op0=ALU.bypass, op1=ALU.mult,
                                accum_out=ct_all[:, b_, kc : kc + 1],
                            )
                        return run

                    def ct_dma():
                        # c_t[b] out: dest viewed [p, kc] (4B-strided, tiny)
                        nc.sync.dma_start(
                            out=ct_out[b_ : b_ + 1, :].rearrange(
                                "o (k p) -> (o p) k", p=P
                            ),
                            in_=ct_all[:, b_, :],
                        )

                    if b_ == BL - 1:
                        return [atn_tp, pe_ct, ct_evict]
                    return (
                        [bcast]
                        + [stt_one(kc) for kc in range(KC)]
                        + [ct_dma]
                    )

                if b == BL - 1:
                    # issue the last batch's tail chain FIRST (the engine
                    # streams are drained ready-first in issue order): its
                    # pass B must not queue behind b-1's leftover DVE work
                    for fn in make_pass_b():
                        fn()
                else:
                    pending_pass_b.extend(make_pass_b())
                    issue_pass_b_one()
                    issue_pass_b_one()

            # keep the PE clock warm through the last batch's eviction and
            # exp waits.  Reading the mt6/mt7 att tiles (ready right after
            # their tanh) pins these into exactly those windows --
            # dependency-free warmups would be hoisted to any earlier idle
            # PE slot by the scheduler.
            ps_tw = psA.tile([P, T], F32, tag="psA")
            for _ in range(5):
                nc.tensor.matmul(
                    ps_tw[:, 0:512], ones_col[:], att_m6[0:1, 0:512],
                    start=True, stop=True,
                )
            for _ in range(4):
                nc.tensor.matmul(
                    ps_tw[:, 0:512], ones_col[:], att_tail[0:1, 0:512],
                    start=True, stop=True,
                )

            while pending_pass_b:
                issue_pass_b_one()

    _legalize_waits(nc)
    return nc


# Walrus rejects instructions whose sync-wait count exceeds the per-opcode
# descriptor slots ("Too many sync wait commands").  Tile can emit 2-3 waits
# on matmuls/DMAs at cross-engine convergence points.  Hoist surplus waits
# onto standalone InstEventSemaphore carriers inserted just before the
# offender in the same engine stream: the engine stalls on the carrier(s),
# then issues the real instruction with a single wait.  Engine streams are
# in-order, so this is semantics-preserving.
_WAIT_SKIP_OPS = {"InstEventSemaphore"}


def _legalize_waits(nc, limit=1):
    import bass_rust

    def make_carrier(engine, wait):
        return mybir.InstNoOp(
            name=nc.get_next_instruction_name(),
            text_hint="waitfix",
            bass_nofuse=True,
            engine=engine,
            sync_info=mybir.SyncInfo(on_wait=[wait], on_update=[]),
        )

    for fn in nc.m.functions:
        for blk in fn.blocks:
            il = blk.instructions
            i = 0
            while i < len(il):
                inst = il[i]
                op = type(inst).__name__
                si = getattr(inst, "sync_info", None)
                if (
                    op in _WAIT_SKIP_OPS
                    or si is None
                    or len(si.on_wait) <= limit
                ):
                    i += 1
                    continue
                waits = list(si.on_wait)
                keep, surplus = waits[-limit:], waits[:-limit]
                carriers = [make_carrier(inst.engine, w) for w in surplus]
                inst.sync_info = bass_rust.SyncInfo(
                    on_wait=keep, on_update=si.on_update
                )
                for k, ev in enumerate(carriers):
                    il.insert(i + k, ev)
                i += len(carriers) + 1


def _get_nc():
    global _NC_CACHE
    if _NC_CACHE is None:
        _NC_CACHE = build_bass()
    return _NC_CACHE


def kernel(s_t_hat, h, enc_padding_mask, coverage, W_h, W_c, dec_W, dec_b, v):
    global LAST_EXEC_NS
    import ml_dtypes

    bf16 = ml_dtypes.bfloat16
    fp8 = ml_dtypes.float8_e4m3
    s_t_hat = np.asarray(s_t_hat, dtype=np.float32)
    h = np.asarray(h, dtype=np.float32)
    enc_padding_mask = np.ascontiguousarray(
        np.asarray(enc_padding_mask, dtype=np.float32)
    )
    coverage = np.ascontiguousarray(np.asarray(coverage, dtype=np.float32))
    W_h = np.asarray(W_h, dtype=np.float32)
    W_c = np.asarray(W_c, dtype=np.float32).reshape(1, N)
    dec_W = np.asarray(dec_W, dtype=np.float32)
    dec_b = np.asarray(dec_b, dtype=np.float32).reshape(1, N)
    v = np.asarray(v, dtype=np.float32)

    hTf = np.transpose(h, (0, 2, 1))  # [B, N, T] fp32 view
    hT = np.ascontiguousarray(hTf.astype(bf16))  # [B, N, T]
    # fp8 chunk pairs: [b, pair, p, c, t] with n = (2*pair + c)*128 + p
    hT8 = np.ascontiguousarray(
        hTf[:, :N8, :].astype(fp8)
        .reshape(B, KC8 // 2, 2, P, T)
        .transpose(0, 1, 3, 2, 4)
    )
    WhTf = W_h.T  # [n, m] fp32
    WhT = np.ascontiguousarray(WhTf[N8:, :].astype(bf16))  # bf16 tail chunks
    WhT8 = np.ascontiguousarray(
        WhTf[:N8, :].astype(fp8)
        .reshape(KC8 // 2, 2, P, N)
        .transpose(0, 2, 1, 3)
    )
    # dec_W.T x16 in fp8, chunk pairs interleaved: [pair, p, c, m] with
    # n = (2*pair + c)*128 + p.  The x16 scale is undone on-device at the
    # dec_feaT eviction; dec_b ships pre-scaled to match.
    decWT8 = np.ascontiguousarray(
        (dec_W.T.astype(np.float32) * 16.0).astype(fp8)
        .reshape(KC // 2, 2, P, N)
        .transpose(0, 2, 1, 3)
    )
    sT = np.ascontiguousarray(s_t_hat.T.astype(bf16))  # [n, B]
    vcol = np.ascontiguousarray(v.reshape(KC, P).T.astype(bf16))  # [p, kc]
    covbc = np.ascontiguousarray(
        np.broadcast_to(
            coverage.astype(bf16)[:, None, :], (B, P, T)
        )
    )  # [B, p, T] cov rows replicated across partitions
    wcT = np.ascontiguousarray(
        W_c.reshape(KC, P).T.astype(np.float32)
    )  # [p, kc]
    decb_b = np.ascontiguousarray((dec_b.astype(np.float32) * 16.0).astype(bf16))

    in_maps = []
    for c in range(NCORES):
        bs = slice(c * BL, (c + 1) * BL)
        in_maps.append(
            {
                "hT": hT[bs],
                "hT8": hT8[bs],
                "hnatl": np.ascontiguousarray(
                    h[(c + 1) * BL - 1].astype(bf16)
                ),
                "maskcol": np.ascontiguousarray(
                    enc_padding_mask[(c + 1) * BL - 1]
                    .reshape(KC, P).T.astype(np.float32)
                ),
                "cov": coverage[bs],
                "covbc": covbc[bs],
                "mask": enc_padding_mask[bs],
                "sT": np.ascontiguousarray(sT[:, bs]),
                "WhT": WhT,
                "WhT8": WhT8,
                "decWT8": decWT8,
                "decb": decb_b,
                "WcT": wcT,
                "vcol": vcol,
            }
        )

    nc = _get_nc()
    trace = os.environ.get("BASS_KERNEL_TRACE", "0") == "1"
    res = run_bass_kernel_spmd(
        nc, in_maps, core_ids=list(range(NCORES)), trace=trace
    )
    LAST_EXEC_NS = res.exec_time_ns

    c_t = np.concatenate([res.results[c]["out_ct"] for c in range(NCORES)], axis=0)
    attn = np.concatenate(
        [res.results[c]["out_attn"] for c in range(NCORES)], axis=0
    )
    cov_new = np.concatenate(
        [res.results[c]["out_cov"] for c in range(NCORES)], axis=0
    )
    return (c_t, attn, cov_new)

